# revision 19
# baseline (speedup 1.0000x reference)
"""Trainium2 Bass kernel: causal multi-head attention with interleaved RoPE.

Problem shapes (hardcoded): x [2, 2048, 1024], 16 heads of dk=64.
Sharding: 8 cores = 2 batches x 4 head-groups (4 heads each). Each core
computes its head-slice Q/K/V projections, RoPE, causal attention, and a
partial output through its Wo row-slice; the host sums the 4 partials per
batch and adds bo.

v2 design (single interleaved pipeline, fp16 operands):
- All matmul operands fp16 (tolerance 2e-2; fp16 keeps ~1e-3).
- Heads processed as 2 pairs per core. Scores for a pair run as two
  concurrent row-tiled matmuls (tile_position rows 0-63 / 64-127) writing
  two PSUM banks in one 512-cycle pass -- 2x score throughput vs padded-K.
- Wo contracts a stacked head-pair (128 rows), halving Wo matmuls.
- One loop over the 4 sq-blocks interleaves projection(j+1) / Wo(j) matmul
  groups into the ACT-bound attention(j) stream so the PE never drains.
- Causal: only the needed q-column range of each diagonal score tile is
  exp'd / PV'd; triangular masking only on the 128-col diagonal band.
- Softmax denominator rides as a 65th "ones" column of V (PV psum row 64);
  reciprocal via DVE reciprocal_approx_fast, broadcast via gpsimd
  partition_broadcast (no PE/PSUM involvement).

RoPE trick: attention scores are invariant to any permutation of the dk
axis applied to both Q and K, so the Wq/Wk columns are permuted on the host
into a "quadrant half-split" layout where each rotation pair partner sits
exactly 16 partitions away inside the same 32-partition quadrant. The DVE
stream_shuffle (a per-quadrant 32-way permute) then produces the swapped
operand, and RoPE becomes: rot = q * cosT + shuffle(q) * sinT with
host-precomputed tables (sinT carries the sign).
"""

from contextlib import ExitStack

import numpy as np

import concourse.bass as bass
import concourse.mybir as mybir
import concourse.tile as tile

B, S, D, H = 2, 2048, 1024, 16
DK = D // H  # 64
HG = 4  # heads per core
NCOLS = HG * DK  # 256 columns of the projection per core
THETA = 10000.0
SCALE = 1.0 / float(np.sqrt(DK))
N_CORES = 8

F32 = mybir.dt.float32
F16 = mybir.dt.float16

SB = 512            # sq block width
NSB = S // SB       # 4
NST = S // 128      # 16 key tiles
NDC = D // 128      # 8 contraction chunks
AUGW = 72           # per-head stride in vaug (64 V cols + ones col + pad)
SLICED = True       # restrict diagonal score tiles to the causal q-cols


def f16(a):
    return np.ascontiguousarray(a, dtype=np.float16)


# ---------------------------------------------------------------------------
# host-side prep
# ---------------------------------------------------------------------------

def _rope_perm():
    """Within-head column permutation pi: new row r -> original dk index."""
    perm = np.empty(DK, dtype=np.int64)
    for r in range(DK):
        q, m = divmod(r, 32)
        if m < 16:
            perm[r] = 2 * (16 * q + m)
        else:
            perm[r] = 2 * (16 * q + m - 16) + 1
    return perm


_PERM = _rope_perm()
SHUF_MASK = list(range(16, 32)) + list(range(16))  # swap 16-halves per quadrant


def _shuf128(v):
    """Apply the quadrant shuffle to a length-128 vector (host mirror)."""
    out = np.empty_like(v)
    for q in range(4):
        out[32 * q:32 * q + 32] = v[32 * q:32 * q + 32][SHUF_MASK]
    return out


def _rope_tables(pos):
    """cosT/sinT [128, S] for the permuted layout. pos: [S] int."""
    inv_freq = (np.float32(THETA) ** (-(np.arange(0, DK, 2, dtype=np.float32) / np.float32(DK))))  # [32]
    ang = pos.astype(np.float32)[:, None] * inv_freq[None, :]  # [S, 32]
    cos = np.cos(ang)  # [S, 32]
    sin = np.sin(ang)
    cosT = np.empty((128, S), dtype=np.float32)
    sinT = np.empty((128, S), dtype=np.float32)
    for p in range(128):
        r = p % DK
        q, m = divmod(r, 32)
        if m < 16:
            i = 16 * q + m
            sgn = -1.0
        else:
            i = 16 * q + m - 16
            sgn = 1.0
        cosT[p] = cos[:, i]
        sinT[p] = np.float32(sgn) * sin[:, i]
    return cosT, sinT


def make_core_inputs(x, token_position, Wq, bq, Wk, bk, Wv, bv, Wo, bo):
    """Build the 8 per-core input maps. Returns (in_maps, with_bias)."""
    x = np.asarray(x, dtype=np.float32)
    token_position = np.asarray(token_position)
    Wq, Wk, Wv, Wo = (np.asarray(w, dtype=np.float32) for w in (Wq, Wk, Wv, Wo))
    bq, bk, bv = (np.asarray(b_, dtype=np.float32) for b_ in (bq, bk, bv))
    with_bias = any(float(np.abs(v).max()) != 0.0 for v in (bq, bk, bv))

    in_maps = []
    tables = {}
    for c in range(N_CORES):
        b, hg = divmod(c, HG)
        heads = range(HG * hg, HG * hg + HG)
        # permuted q/k column indices for this core's heads
        cols_qk = np.concatenate([DK * h + _PERM for h in heads])
        cols_v = np.arange(NCOLS * hg, NCOLS * hg + NCOLS)
        if b not in tables:
            tables[b] = _rope_tables(np.asarray(token_position[b]))
        cosT, sinT = tables[b]
        # Wo rows stacked per head pair: wo[r, p, :] = Wo row of
        # (local head 2p + r//64, dk r%64)
        wo_rows = Wo[cols_v, :].reshape(2, 2 * DK, D)  # [pair, 128, D]
        m = {
            "xT": f16(x[b].T),                               # [1024, 2048]
            "wq": f16(Wq[:, cols_qk]),                       # [1024, 256]
            "wk": f16(Wk[:, cols_qk]),
            "wv": f16(Wv[:, cols_v]),
            "wo": f16(wo_rows.transpose(1, 0, 2)),           # [128, 2, 1024]
            "cosT": f16(cosT),
            "sinT": f16(sinT),
        }
        if with_bias:
            # rope is linear: rope(q + b) = rope(q) + rope(b); rope(b) is a
            # position-dependent table added after the rope combine.
            for nm, bb in (("bqr", bq[cols_qk]), ("bkr", bk[cols_qk])):
                chunks = []
                for half in range(2):
                    blk = bb[128 * half:128 * half + 128]  # [128]
                    tbl = blk[:, None] * cosT + _shuf128(blk)[:, None] * sinT
                    chunks.append(f16(tbl))
                m[nm] = np.stack(chunks, axis=0)  # [2, 128, S]
            m["bvb"] = f16(np.broadcast_to(bv[cols_v][None, :], (128, NCOLS)).copy())
        in_maps.append(m)
    return in_maps, with_bias


# ---------------------------------------------------------------------------
# device program
# ---------------------------------------------------------------------------

def build_program(with_bias=False, dbg=False):
    from concourse import bacc, library_config
    nc = bacc.Bacc("TRN2", debug=False)

    xT = nc.declare_dram_parameter("xT", [D, S], F16, isOutput=False).ap()
    wq = nc.declare_dram_parameter("wq", [D, NCOLS], F16, isOutput=False).ap()
    wk = nc.declare_dram_parameter("wk", [D, NCOLS], F16, isOutput=False).ap()
    wv = nc.declare_dram_parameter("wv", [D, NCOLS], F16, isOutput=False).ap()
    wo = nc.declare_dram_parameter("wo", [128, 2, D], F16, isOutput=False).ap()
    cosT = nc.declare_dram_parameter("cosT", [128, S], F16, isOutput=False).ap()
    sinT = nc.declare_dram_parameter("sinT", [128, S], F16, isOutput=False).ap()
    if with_bias:
        bqr = nc.declare_dram_parameter("bqr", [2, 128, S], F16, isOutput=False).ap()
        bkr = nc.declare_dram_parameter("bkr", [2, 128, S], F16, isOutput=False).ap()
        bvb = nc.declare_dram_parameter("bvb", [128, NCOLS], F16, isOutput=False).ap()
    out = nc.declare_dram_parameter("out", [S, D], F16, isOutput=True).ap()
    if dbg:
        dbg_t = {}
        for nm, shp, dt in (("dbg_qt", [128, SB], F16), ("dbg_kt", [128, SB], F16),
                            ("dbg_va", [128, HG * AUGW], F16),
                            ("dbg_e", [128, 2 * SB], F16),
                            ("dbg_den", [128, SB], F32),
                            ("dbg_rec", [128, SB], F32),
                            ("dbg_rbc", [128, SB], F32),
                            ("dbg_ot", [128, SB], F16)):
            dbg_t[nm] = nc.declare_dram_parameter(nm, shp, dt, isOutput=True).ap()

    with tile.TileContext(nc) as tc, ExitStack() as ctx:
        nc.gpsimd.load_library(library_config.proxy)
        const = ctx.enter_context(tc.tile_pool(name="const", bufs=1))
        sbig = ctx.enter_context(tc.tile_pool(name="sbig", bufs=1))
        xts = ctx.enter_context(tc.tile_pool(name="xts", bufs=16))
        rtmp = ctx.enter_context(tc.tile_pool(name="rtmp", bufs=2))
        epool = ctx.enter_context(tc.tile_pool(name="epool", bufs=3))
        npool = ctx.enter_context(tc.tile_pool(name="npool", bufs=2))
        opool = ctx.enter_context(tc.tile_pool(name="opool", bufs=3))
        ps_big = ctx.enter_context(tc.tile_pool(name="ps_big", bufs=2, space="PSUM"))
        ps_pv = ctx.enter_context(tc.tile_pool(name="ps_pv", bufs=2, space="PSUM"))
        ps_pj = ctx.enter_context(tc.tile_pool(name="ps_pj", bufs=2, space="PSUM"))

        # --- weights / tables resident in SBUF (per-dc tiles: finer deps,
        # so the first projection matmuls start after ~64KB of DMA).
        # DMA priority order: wq/wk (first QK groups), then x block 0 (issued
        # in emit_proj(0) below), then cos/sin (rope), wv, wo.
        wq_sb = [const.tile([128, NCOLS], F16, tag=f"wq{dc}", name=f"wq{dc}")
                 for dc in range(NDC)]
        wk_sb = [const.tile([128, NCOLS], F16, tag=f"wk{dc}", name=f"wk{dc}")
                 for dc in range(NDC)]
        wv_sb = [const.tile([128, NCOLS], F16, tag=f"wv{dc}", name=f"wv{dc}")
                 for dc in range(NDC)]
        for dc in range(NDC):
            nc.sync.dma_start(wq_sb[dc][:], wq[128 * dc:128 * dc + 128, :])
            nc.sync.dma_start(wk_sb[dc][:], wk[128 * dc:128 * dc + 128, :])
        cos_sb = const.tile([128, S], F16, tag="cos")
        sin_sb = const.tile([128, S], F16, tag="sin")
        wo_sb = const.tile([128, 2, D], F16, tag="wo")
        if with_bias:
            bqr_t = [const.tile([128, S], F16, tag=f"bqr{c}", name=f"bqr{c}")
                     for c in range(2)]
            bkr_t = [const.tile([128, S], F16, tag=f"bkr{c}", name=f"bkr{c}")
                     for c in range(2)]
            bvb_sb = const.tile([128, NCOLS], F16, tag="bvb")

        # rope'd K^T per (chunk, block); resident for the whole sequence
        kth = [[sbig.tile([128, SB], F16, tag=f"kh{c}_{sb}", name=f"kh{c}_{sb}")
                for sb in range(NSB)] for c in range(2)]
        # V augmented with a ones column per head, per key tile
        vaug = [sbig.tile([128, HG * AUGW], F16, tag=f"va{st}", name=f"va{st}")
                for st in range(NST)]

        # ------------------------------------------------------------------
        # helpers
        # ------------------------------------------------------------------

        def emit_proj(j):
            """Allocate block-j projection outputs and return (qt_pair, ops):
            ops is a list of closures, each issuing one PE matmul group plus
            its vector consumers."""
            ss = slice(SB * j, SB * j + SB)
            xt_t = []
            for dc in range(NDC):
                t = xts.tile([128, SB], F16, tag="xt", name=f"xt{j}_{dc}")
                nc.sync.dma_start(t[:], xT[128 * dc:128 * dc + 128, ss])
                xt_t.append(t)
            # rope'd Q^T pair tiles for this block (2-buf rotation)
            qt_pair = [sbig.tile([128, SB], F16, tag=f"qt{c}", name=f"qt{j}_{c}",
                                 bufs=2) for c in range(2)]

            ops = []

            def qk_group(c, kind):
                def run():
                    w_sb = wq_sb if kind == "q" else wk_sb
                    ncol = slice(128 * c, 128 * c + 128)
                    ps = ps_pj.tile([128, SB], F32, tag="pj", name="psqk")
                    for dc in range(NDC):
                        nc.tensor.matmul(ps[:], w_sb[dc][:, ncol], xt_t[dc][:],
                                         start=(dc == 0), stop=(dc == NDC - 1))
                    # rope: dst = ps*cos + shuffle(ps)*sin (+ bias table)
                    t_cos = rtmp.tile([128, SB], F16, tag="rc", name="tcos")
                    nc.vector.tensor_mul(t_cos[:], ps[:], cos_sb[:, ss])
                    t_shuf = rtmp.tile([128, SB], F32, tag="rs", name="tshuf")
                    nc.vector.stream_shuffle(t_shuf[:], ps[:], SHUF_MASK)
                    t_sin = rtmp.tile([128, SB], F16, tag="rm", name="tsin")
                    nc.gpsimd.tensor_mul(t_sin[:], t_shuf[:], sin_sb[:, ss])
                    dst = qt_pair[c] if kind == "q" else kth[c][j]
                    nc.vector.tensor_add(dst[:], t_cos[:], t_sin[:])
                    if with_bias:
                        bt = bqr_t[c] if kind == "q" else bkr_t[c]
                        nc.vector.tensor_add(dst[:], dst[:], bt[:, ss])
                return run

            def v_group(st4):
                def run():
                    st = 4 * j + st4
                    ps = ps_pj.tile([128, SB], F32, tag="pj", name="psv")
                    for dc in range(NDC):
                        nc.tensor.matmul(ps[:, 0:NCOLS],
                                         xt_t[dc][:, 128 * st4:128 * st4 + 128],
                                         wv_sb[dc][:],
                                         start=(dc == 0), stop=(dc == NDC - 1))
                    va = vaug[st][:].rearrange("p (h e) -> p h e", h=HG)
                    psv = ps[:, 0:NCOLS].rearrange("p (h k) -> p h k", h=HG)
                    if with_bias:
                        bvv = bvb_sb[:].rearrange("p (h k) -> p h k", h=HG)
                        nc.vector.tensor_add(va[:, :, 0:DK], psv, bvv)
                    else:
                        nc.vector.tensor_copy(va[:, :, 0:DK], psv)
                return run

            # chunk-0 attention deps first, then V, then chunk 1
            ops.append(qk_group(0, "k"))
            ops.append(qk_group(0, "q"))
            for st4 in range(4):
                ops.append(v_group(st4))
            ops.append(qk_group(1, "k"))
            ops.append(qk_group(1, "q"))
            return qt_pair, ops

        def emit_wo(j, ot_pair):
            """Wo matmul groups for block j (reads the ot pair tiles)."""
            def grp(rq):
                def run():
                    ps = ps_big.tile([128, 2 * SB], F32, tag="sc", name="pswo")
                    for half in range(2):
                        cols = slice(SB * half, SB * half + SB)
                        nc.tensor.matmul(ps[:, cols],
                                         ot_pair[0][:, 128 * rq:128 * rq + 128],
                                         wo_sb[:, 0, cols], start=True, stop=False)
                        nc.tensor.matmul(ps[:, cols],
                                         ot_pair[1][:, 128 * rq:128 * rq + 128],
                                         wo_sb[:, 1, cols], start=False, stop=True)
                    o_sb = opool.tile([128, 2 * SB], F16, tag="osb", name="osb")
                    r0 = SB * j + 128 * rq
                    # split copy+DMA into halves for finer overlap; output DMA
                    # on the gpsimd queue so it never head-of-line blocks the
                    # input prefetches on the sync queue
                    for hf in range(2):
                        cl = slice(SB * hf, SB * hf + SB)
                        nc.vector.tensor_copy(o_sb[:, cl], ps[:, cl])
                        nc.gpsimd.dma_start(out[r0:r0 + 128, cl], o_sb[:, cl])
                return run

            return [grp(rq) for rq in range(4)]

        # ------------------------------------------------------------------
        # main pipeline over sq blocks
        # ------------------------------------------------------------------

        # filler: list of (due_block, closure). A filler op must have run
        # before the attention of block `due_block` is emitted.
        filler = []

        qt_pair, ops = emit_proj(0)  # also issues block-0 x DMAs
        # block-0 critical path first: wv (V proj), block-0 rope tables
        for dc in range(NDC):
            nc.sync.dma_start(wv_sb[dc][:], wv[128 * dc:128 * dc + 128, :])
        nc.sync.dma_start(cos_sb[:, 0:SB], cosT[:, 0:SB])
        nc.sync.dma_start(sin_sb[:, 0:SB], sinT[:, 0:SB])
        # remaining table columns + Wo afterwards
        nc.sync.dma_start(cos_sb[:, SB:S], cosT[:, SB:S])
        nc.sync.dma_start(sin_sb[:, SB:S], sinT[:, SB:S])
        nc.sync.dma_start(wo_sb[:], wo)
        if with_bias:
            for c in range(2):
                nc.sync.dma_start(bqr_t[c][:], bqr[c])
                nc.sync.dma_start(bkr_t[c][:], bkr[c])
            nc.sync.dma_start(bvb_sb[:], bvb)
        for st in range(NST):
            va = vaug[st][:].rearrange("p (h e) -> p h e", h=HG)
            nc.gpsimd.memset(va[:, :, DK], 1.0)
        for op in ops:  # block 0 projections run up front
            op()

        for j in range(NSB):
            # anything due by this block runs now (normally already drained)
            while filler and filler[0][0] <= j:
                filler.pop(0)[1]()
            if j + 1 < NSB:
                nqt, ops = emit_proj(j + 1)
                filler.extend((j + 1, op) for op in ops)
            else:
                nqt = None

            ot_pair = [sbig.tile([128, SB], F16, tag=f"ot{c}", name=f"ot{j}_{c}",
                                 bufs=2) for c in range(2)]

            for c in range(2):
                pvA = ps_pv.tile([128, SB], F32, tag="pv", name="pvA")
                pvB = ps_pv.tile([128, SB], F32, tag="pv", name="pvB")
                n_tiles = 4 * j + 4
                for i in range(n_tiles):
                    kb, t = divmod(i, 4)
                    m = i - 4 * j
                    # q columns < c0 are entirely above the causal diagonal
                    c0 = 128 * m if (SLICED and m > 0) else 0
                    kt = kth[c][kb]
                    sc = ps_big.tile([128, 2 * SB], F32, tag="sc", name="sc")
                    nc.tensor.matmul(sc[:, c0:SB],
                                     kt[0:64, 128 * t:128 * t + 128],
                                     qt_pair[c][0:64, c0:SB],
                                     start=True, stop=True)
                    nc.tensor.matmul(sc[:, SB + c0:2 * SB],
                                     kt[64:128, 128 * t:128 * t + 128],
                                     qt_pair[c][64:128, c0:SB],
                                     start=True, stop=True)
                    e = epool.tile([128, 2 * SB], F16, tag="e", name="e")
                    ev = e[:].rearrange("p (g f) -> p g f", g=2)
                    scv = sc[:].rearrange("p (g f) -> p g f", g=2)
                    nc.scalar.activation(ev[:, :, c0:SB], scv[:, :, c0:SB],
                                         mybir.ActivationFunctionType.Exp,
                                         scale=SCALE)
                    if m >= 0:  # diagonal tile: zero the triangular band
                        band = ev[:, :, c0:128 * (m + 1)]
                        nc.gpsimd.affine_select(
                            out=band, in_=band,
                            compare_op=mybir.AluOpType.is_ge,
                            fill=0.0, base=c0 - 128 * m,
                            pattern=[[0, 2], [1, 128 * (m + 1) - c0]],
                            channel_multiplier=-1)
                    if dbg and j == 0 and c == 0 and i == 0:
                        nc.sync.dma_start(dbg_t["dbg_e"][:], e[:])
                    va = vaug[i][:].rearrange("p (h e) -> p h e", h=HG)
                    nc.tensor.matmul(pvA[0:DK + 1, c0:SB],
                                     va[:, 2 * c, 0:DK + 1], e[:, c0:SB],
                                     start=(i == 0), stop=(i == n_tiles - 1))
                    nc.tensor.matmul(pvB[0:DK + 1, c0:SB],
                                     va[:, 2 * c + 1, 0:DK + 1],
                                     e[:, SB + c0:2 * SB],
                                     start=(i == 0), stop=(i == n_tiles - 1))
                    # drain one filler PE group per key tile
                    if filler:
                        filler.pop(0)[1]()
                # normalize: ot rows = pv[0:64] * broadcast(1/pv[64])
                for half, pv in ((0, pvA), (1, pvB)):
                    # custom-DVE ops misread PSUM at base_partition 64 on HW:
                    # stage the denominator row to SBUF partition 0 first
                    dsb = npool.tile([1, SB], F32, tag="dsb", name="dsb")
                    nc.vector.tensor_copy(dsb[0:1, :], pv[DK:DK + 1, :])
                    rec = npool.tile([1, SB], F32, tag="rec", name="rec")
                    nc.vector.reciprocal_approx_fast(rec[0:1, :], dsb[0:1, :])
                    rbc = npool.tile([DK, SB], F32, tag="rbc", name="rbc")
                    nc.gpsimd.partition_broadcast(rbc[0:DK, :], rec[0:1, :])
                    rows = slice(DK * half, DK * half + DK)
                    if dbg and j == 0 and c == 0 and half == 0:
                        den = npool.tile([1, SB], F32, tag="den", name="den")
                        nc.vector.tensor_copy(den[0:1, :], pv[DK:DK + 1, :])
                        nc.sync.dma_start(dbg_t["dbg_den"][0:1, :], den[0:1, :])
                        nc.sync.dma_start(dbg_t["dbg_rec"][0:1, :], rec[0:1, :])
                        nc.sync.dma_start(dbg_t["dbg_rbc"][0:DK, :], rbc[0:DK, :])
                    nc.vector.tensor_mul(ot_pair[c][rows, :], pv[0:DK, :],
                                         rbc[0:DK, :])
                if dbg and j == 0 and c == 0:
                    nc.sync.dma_start(dbg_t["dbg_qt"][:], qt_pair[0][:])
                    nc.sync.dma_start(dbg_t["dbg_kt"][:], kth[0][0][:])
                    nc.sync.dma_start(dbg_t["dbg_va"][:], vaug[0][:])
                    nc.sync.dma_start(dbg_t["dbg_ot"][:], ot_pair[0][:])

            # Wo for this block; on the last block run immediately, else queue
            wops = emit_wo(j, ot_pair)
            if j + 1 == NSB:
                for _, op in filler:
                    op()
                for op in wops:
                    op()
                filler = []
            else:
                filler.extend((j + 2, op) for op in wops)
            qt_pair = nqt

    nc.compile()
    return nc


_CACHED_NC = {}


def _get_program(with_bias=False):
    if with_bias not in _CACHED_NC:
        _CACHED_NC[with_bias] = build_program(with_bias=with_bias)
    return _CACHED_NC[with_bias]


# ---------------------------------------------------------------------------
# entry point
# ---------------------------------------------------------------------------

def kernel(x, token_position, Wq, bq, Wk, bk, Wv, bv, Wo, bo, _results=None):
    from concourse.bass_utils import run_bass_kernel_spmd

    in_maps, with_bias = make_core_inputs(
        x, token_position, Wq, bq, Wk, bk, Wv, bv, Wo, bo)
    if _results is None:
        nc = _get_program(with_bias=with_bias)
        res = run_bass_kernel_spmd(nc, in_maps, list(range(N_CORES)))
        _results = [res.results[i]["out"] for i in range(N_CORES)]
    bo = np.asarray(bo, dtype=np.float32)
    out = np.empty((B, S, D), dtype=np.float32)
    for b in range(B):
        acc = _results[HG * b].astype(np.float32)
        for hg in range(1, HG):
            acc = acc + _results[HG * b + hg].astype(np.float32)
        out[b] = acc + bo[None, :]
    return out


# revision 30
# speedup vs baseline: 1.0107x; 1.0107x over previous
"""Trainium2 Bass kernel: causal multi-head attention with interleaved RoPE.

Problem shapes (hardcoded): x [2, 2048, 1024], 16 heads of dk=64.
Sharding: 8 cores = 2 batches x 4 head-groups (4 heads each). Each core
computes its head-slice Q/K/V projections, RoPE, causal attention, and a
partial output through its Wo row-slice; the host sums the 4 partials per
batch and adds bo.

v2 design (single interleaved pipeline, fp16 operands):
- All matmul operands fp16 (tolerance 2e-2; fp16 keeps ~1e-3).
- Heads processed as 2 pairs per core. Scores for a pair run as two
  concurrent row-tiled matmuls (tile_position rows 0-63 / 64-127) writing
  two PSUM banks in one 512-cycle pass -- 2x score throughput vs padded-K.
- Wo contracts a stacked head-pair (128 rows), halving Wo matmuls.
- One loop over the 4 sq-blocks interleaves projection(j+1) / Wo(j) matmul
  groups into the ACT-bound attention(j) stream so the PE never drains.
- Causal: only the needed q-column range of each diagonal score tile is
  exp'd / PV'd; triangular masking only on the 128-col diagonal band.
- Softmax denominator rides as a 65th "ones" column of V (PV psum row 64);
  reciprocal via DVE reciprocal_approx_fast, broadcast via gpsimd
  partition_broadcast (no PE/PSUM involvement).

RoPE trick: attention scores are invariant to any permutation of the dk
axis applied to both Q and K, so the Wq/Wk columns are permuted on the host
into a "quadrant half-split" layout where each rotation pair partner sits
exactly 16 partitions away inside the same 32-partition quadrant. The DVE
stream_shuffle (a per-quadrant 32-way permute) then produces the swapped
operand, and RoPE becomes: rot = q * cosT + shuffle(q) * sinT with
host-precomputed tables (sinT carries the sign).
"""

from contextlib import ExitStack

import numpy as np

import concourse.bass as bass
import concourse.mybir as mybir
import concourse.tile as tile

B, S, D, H = 2, 2048, 1024, 16
DK = D // H  # 64
HG = 4  # heads per core
NCOLS = HG * DK  # 256 columns of the projection per core
THETA = 10000.0
SCALE = 1.0 / float(np.sqrt(DK))
N_CORES = 8

F32 = mybir.dt.float32
F16 = mybir.dt.float16

SB = 512            # sq block width
NSB = S // SB       # 4
NST = S // 128      # 16 key tiles
NDC = D // 128      # 8 contraction chunks
AUGW = 72           # per-head stride in vaug (64 V cols + ones col + pad)
SLICED = True       # restrict diagonal score tiles to the causal q-cols


def f16(a):
    return np.ascontiguousarray(a, dtype=np.float16)


# ---------------------------------------------------------------------------
# host-side prep
# ---------------------------------------------------------------------------

def _rope_perm():
    """Within-head column permutation pi: new row r -> original dk index."""
    perm = np.empty(DK, dtype=np.int64)
    for r in range(DK):
        q, m = divmod(r, 32)
        if m < 16:
            perm[r] = 2 * (16 * q + m)
        else:
            perm[r] = 2 * (16 * q + m - 16) + 1
    return perm


_PERM = _rope_perm()
SHUF_MASK = list(range(16, 32)) + list(range(16))  # swap 16-halves per quadrant


def _shuf128(v):
    """Apply the quadrant shuffle to a length-128 vector (host mirror)."""
    out = np.empty_like(v)
    for q in range(4):
        out[32 * q:32 * q + 32] = v[32 * q:32 * q + 32][SHUF_MASK]
    return out


def _rope_tables(pos):
    """cosT/sinT [128, S] for the permuted layout. pos: [S] int."""
    inv_freq = (np.float32(THETA) ** (-(np.arange(0, DK, 2, dtype=np.float32) / np.float32(DK))))  # [32]
    ang = pos.astype(np.float32)[:, None] * inv_freq[None, :]  # [S, 32]
    cos = np.cos(ang)  # [S, 32]
    sin = np.sin(ang)
    cosT = np.empty((128, S), dtype=np.float32)
    sinT = np.empty((128, S), dtype=np.float32)
    for p in range(128):
        r = p % DK
        q, m = divmod(r, 32)
        if m < 16:
            i = 16 * q + m
            sgn = -1.0
        else:
            i = 16 * q + m - 16
            sgn = 1.0
        cosT[p] = cos[:, i]
        sinT[p] = np.float32(sgn) * sin[:, i]
    return cosT, sinT


def make_core_inputs(x, token_position, Wq, bq, Wk, bk, Wv, bv, Wo, bo):
    """Build the 8 per-core input maps. Returns (in_maps, with_bias)."""
    x = np.asarray(x, dtype=np.float32)
    token_position = np.asarray(token_position)
    Wq, Wk, Wv, Wo = (np.asarray(w, dtype=np.float32) for w in (Wq, Wk, Wv, Wo))
    bq, bk, bv = (np.asarray(b_, dtype=np.float32) for b_ in (bq, bk, bv))
    with_bias = any(float(np.abs(v).max()) != 0.0 for v in (bq, bk, bv))

    in_maps = []
    tables = {}
    for c in range(N_CORES):
        b, hg = divmod(c, HG)
        heads = range(HG * hg, HG * hg + HG)
        # permuted q/k column indices for this core's heads
        cols_qk = np.concatenate([DK * h + _PERM for h in heads])
        cols_v = np.arange(NCOLS * hg, NCOLS * hg + NCOLS)
        if b not in tables:
            tables[b] = _rope_tables(np.asarray(token_position[b]))
        cosT, sinT = tables[b]
        # Wo rows stacked per head pair: wo[r, p, :] = Wo row of
        # (local head 2p + r//64, dk r%64)
        wo_rows = Wo[cols_v, :].reshape(2, 2 * DK, D)  # [pair, 128, D]
        # weights/x pre-laid in SBUF layout [128, dc, cols] so each tensor
        # loads with a single large DMA (per-DMA queue issue is ~650ns)
        def dc_layout(w):  # [1024, C] -> [128, 8, C]
            return f16(w.reshape(NDC, 128, -1).transpose(1, 0, 2))
        m = {
            "xT": dc_layout(x[b].T),                         # [128, 8, 2048]
            "wq": dc_layout(Wq[:, cols_qk]),                 # [128, 8, 256]
            "wk": dc_layout(Wk[:, cols_qk]),
            "wv": dc_layout(Wv[:, cols_v]),
            "wo": f16(wo_rows.transpose(1, 0, 2)),           # [128, 2, 1024]
            "cosT": f16(cosT),
            "sinT": f16(sinT),
        }
        if with_bias:
            # rope is linear: rope(q + b) = rope(q) + rope(b); rope(b) is a
            # position-dependent table added after the rope combine.
            for nm, bb in (("bqr", bq[cols_qk]), ("bkr", bk[cols_qk])):
                chunks = []
                for half in range(2):
                    blk = bb[128 * half:128 * half + 128]  # [128]
                    tbl = blk[:, None] * cosT + _shuf128(blk)[:, None] * sinT
                    chunks.append(f16(tbl))
                m[nm] = np.stack(chunks, axis=0)  # [2, 128, S]
            m["bvb"] = f16(np.broadcast_to(bv[cols_v][None, :], (128, NCOLS)).copy())
        in_maps.append(m)
    return in_maps, with_bias


# ---------------------------------------------------------------------------
# device program
# ---------------------------------------------------------------------------

def build_program(with_bias=False, dbg=False):
    from concourse import bacc, library_config
    nc = bacc.Bacc("TRN2", debug=False)

    xT = nc.declare_dram_parameter("xT", [128, NDC, S], F16, isOutput=False).ap()
    wq = nc.declare_dram_parameter("wq", [128, NDC, NCOLS], F16, isOutput=False).ap()
    wk = nc.declare_dram_parameter("wk", [128, NDC, NCOLS], F16, isOutput=False).ap()
    wv = nc.declare_dram_parameter("wv", [128, NDC, NCOLS], F16, isOutput=False).ap()
    wo = nc.declare_dram_parameter("wo", [128, 2, D], F16, isOutput=False).ap()
    cosT = nc.declare_dram_parameter("cosT", [128, S], F16, isOutput=False).ap()
    sinT = nc.declare_dram_parameter("sinT", [128, S], F16, isOutput=False).ap()
    if with_bias:
        bqr = nc.declare_dram_parameter("bqr", [2, 128, S], F16, isOutput=False).ap()
        bkr = nc.declare_dram_parameter("bkr", [2, 128, S], F16, isOutput=False).ap()
        bvb = nc.declare_dram_parameter("bvb", [128, NCOLS], F16, isOutput=False).ap()
    out = nc.declare_dram_parameter("out", [S, D], F16, isOutput=True).ap()
    if dbg:
        dbg_t = {}
        for nm, shp, dt in (("dbg_qt", [128, SB], F16), ("dbg_kt", [128, SB], F16),
                            ("dbg_va", [128, HG * AUGW], F16),
                            ("dbg_e", [128, 2 * SB], F16),
                            ("dbg_den", [128, SB], F32),
                            ("dbg_rec", [128, SB], F32),
                            ("dbg_rbc", [128, SB], F32),
                            ("dbg_ot", [128, SB], F16)):
            dbg_t[nm] = nc.declare_dram_parameter(nm, shp, dt, isOutput=True).ap()

    with tile.TileContext(nc) as tc, ExitStack() as ctx:
        nc.gpsimd.load_library(library_config.proxy)
        const = ctx.enter_context(tc.tile_pool(name="const", bufs=1))
        sbig = ctx.enter_context(tc.tile_pool(name="sbig", bufs=1))
        xts = ctx.enter_context(tc.tile_pool(name="xts", bufs=2))
        rtmp = ctx.enter_context(tc.tile_pool(name="rtmp", bufs=2))
        epool = ctx.enter_context(tc.tile_pool(name="epool", bufs=3))
        npool = ctx.enter_context(tc.tile_pool(name="npool", bufs=2))
        opool = ctx.enter_context(tc.tile_pool(name="opool", bufs=3))
        ps_big = ctx.enter_context(tc.tile_pool(name="ps_big", bufs=2, space="PSUM"))
        ps_pv = ctx.enter_context(tc.tile_pool(name="ps_pv", bufs=2, space="PSUM"))
        ps_pj = ctx.enter_context(tc.tile_pool(name="ps_pj", bufs=2, space="PSUM"))

        # --- weights / tables resident in SBUF, one large DMA per tensor
        # (per-DMA queue issue costs ~650ns; batching beats fine deps).
        # Priority order: wk/wq, x block 0 (issued in emit_proj(0) below),
        # wv, block-0 rope table columns, the rest.
        wq_sb = const.tile([128, NDC, NCOLS], F16, tag="wq")
        wk_sb = const.tile([128, NDC, NCOLS], F16, tag="wk")
        wv_sb = const.tile([128, NDC, NCOLS], F16, tag="wv")
        nc.sync.dma_start(wk_sb[:], wk)
        nc.sync.dma_start(wq_sb[:], wq)
        # HAM warm-up: dep-free junk matmuls keep the PE clock at 2.4 GHz
        # through the initial DMA fill (idle >3.4us re-throttles to 1.2 GHz)
        warm = const.tile([128, SB], F16, tag="warm")
        nc.gpsimd.memset(warm[:], 0.0)
        wps = ps_pj.tile([128, SB], F32, tag="pj", name="warmps")
        for w in range(16):
            nc.tensor.matmul(wps[:], warm[:, 0:128], warm[:],
                             start=(w == 0), stop=(w == 15))
        cos_sb = const.tile([128, S], F16, tag="cos")
        sin_sb = const.tile([128, S], F16, tag="sin")
        wo_sb = const.tile([128, 2, D], F16, tag="wo")
        if with_bias:
            bqr_t = [const.tile([128, S], F16, tag=f"bqr{c}", name=f"bqr{c}")
                     for c in range(2)]
            bkr_t = [const.tile([128, S], F16, tag=f"bkr{c}", name=f"bkr{c}")
                     for c in range(2)]
            bvb_sb = const.tile([128, NCOLS], F16, tag="bvb")

        # rope'd K^T per (chunk, block); resident for the whole sequence
        kth = [[sbig.tile([128, SB], F16, tag=f"kh{c}_{sb}", name=f"kh{c}_{sb}")
                for sb in range(NSB)] for c in range(2)]
        # V augmented with a ones column per head, per key tile
        vaug = [sbig.tile([128, HG * AUGW], F16, tag=f"va{st}", name=f"va{st}")
                for st in range(NST)]

        # ------------------------------------------------------------------
        # helpers
        # ------------------------------------------------------------------

        def emit_proj(j):
            """Allocate block-j projection outputs and return (qt_pair, ops):
            ops is a list of closures, each issuing one PE matmul group plus
            its vector consumers."""
            ss = slice(SB * j, SB * j + SB)
            xt_t = xts.tile([128, NDC, SB], F16, tag="xt", name=f"xt{j}")
            nc.sync.dma_start(xt_t[:], xT[:, :, ss])
            # rope'd Q^T pair tiles for this block (2-buf rotation)
            qt_pair = [sbig.tile([128, SB], F16, tag=f"qt{c}", name=f"qt{j}_{c}",
                                 bufs=2) for c in range(2)]

            ops = []

            def qk_group(c, kind):
                def run():
                    w_sb = wq_sb if kind == "q" else wk_sb
                    ncol = slice(128 * c, 128 * c + 128)
                    ps = ps_pj.tile([128, SB], F32, tag="pj", name="psqk")
                    for dc in range(NDC):
                        nc.tensor.matmul(ps[:], w_sb[:, dc, ncol],
                                         xt_t[:, dc, :],
                                         start=(dc == 0), stop=(dc == NDC - 1))
                    # rope: dst = ps*cos + shuffle(ps)*sin (+ bias table)
                    t_cos = rtmp.tile([128, SB], F16, tag="rc", name="tcos")
                    nc.vector.tensor_mul(t_cos[:], ps[:], cos_sb[:, ss])
                    t_shuf = rtmp.tile([128, SB], F32, tag="rs", name="tshuf")
                    nc.vector.stream_shuffle(t_shuf[:], ps[:], SHUF_MASK)
                    t_sin = rtmp.tile([128, SB], F16, tag="rm", name="tsin")
                    nc.gpsimd.tensor_mul(t_sin[:], t_shuf[:], sin_sb[:, ss])
                    dst = qt_pair[c] if kind == "q" else kth[c][j]
                    nc.vector.tensor_add(dst[:], t_cos[:], t_sin[:])
                    if with_bias:
                        bt = bqr_t[c] if kind == "q" else bkr_t[c]
                        nc.vector.tensor_add(dst[:], dst[:], bt[:, ss])
                return run

            def v_group(st4):
                def run():
                    st = 4 * j + st4
                    ps = ps_pj.tile([128, SB], F32, tag="pj", name="psv")
                    for dc in range(NDC):
                        nc.tensor.matmul(ps[:, 0:NCOLS],
                                         xt_t[:, dc, 128 * st4:128 * st4 + 128],
                                         wv_sb[:, dc, :],
                                         start=(dc == 0), stop=(dc == NDC - 1))
                    va = vaug[st][:].rearrange("p (h e) -> p h e", h=HG)
                    psv = ps[:, 0:NCOLS].rearrange("p (h k) -> p h k", h=HG)
                    if with_bias:
                        bvv = bvb_sb[:].rearrange("p (h k) -> p h k", h=HG)
                        nc.vector.tensor_add(va[:, :, 0:DK], psv, bvv)
                    else:
                        nc.vector.tensor_copy(va[:, :, 0:DK], psv)
                return run

            # chunk-0 attention deps first, then V, then chunk 1
            ops.append(qk_group(0, "k"))
            ops.append(qk_group(0, "q"))
            for st4 in range(4):
                ops.append(v_group(st4))
            ops.append(qk_group(1, "k"))
            ops.append(qk_group(1, "q"))
            return qt_pair, ops

        def emit_wo(j, ot_pair):
            """Wo matmul groups for block j (reads the ot pair tiles)."""
            def grp(rq):
                def run():
                    ps = ps_big.tile([128, 2 * SB], F32, tag="sc", name="pswo")
                    for half in range(2):
                        cols = slice(SB * half, SB * half + SB)
                        nc.tensor.matmul(ps[:, cols],
                                         ot_pair[0][:, 128 * rq:128 * rq + 128],
                                         wo_sb[:, 0, cols], start=True, stop=False)
                        nc.tensor.matmul(ps[:, cols],
                                         ot_pair[1][:, 128 * rq:128 * rq + 128],
                                         wo_sb[:, 1, cols], start=False, stop=True)
                    o_sb = opool.tile([128, 2 * SB], F16, tag="osb", name="osb")
                    nc.vector.tensor_copy(o_sb[:], ps[:])
                    r0 = SB * j + 128 * rq
                    nc.sync.dma_start(out[r0:r0 + 128, :], o_sb[:])
                return run

            return [grp(rq) for rq in range(4)]

        # ------------------------------------------------------------------
        # main pipeline over sq blocks
        # ------------------------------------------------------------------

        # filler: list of (due_block, closure). A filler op must have run
        # before the attention of block `due_block` is emitted.
        filler = []

        qt_pair, ops = emit_proj(0)  # also issues block-0 x DMAs
        # block-0 critical path first: wv (V proj), block-0 rope tables
        nc.sync.dma_start(wv_sb[:], wv)
        nc.sync.dma_start(cos_sb[:, 0:SB], cosT[:, 0:SB])
        nc.sync.dma_start(sin_sb[:, 0:SB], sinT[:, 0:SB])
        # remaining table columns + Wo afterwards
        nc.sync.dma_start(cos_sb[:, SB:S], cosT[:, SB:S])
        nc.sync.dma_start(sin_sb[:, SB:S], sinT[:, SB:S])
        nc.sync.dma_start(wo_sb[:], wo)
        if with_bias:
            for c in range(2):
                nc.sync.dma_start(bqr_t[c][:], bqr[c])
                nc.sync.dma_start(bkr_t[c][:], bkr[c])
            nc.sync.dma_start(bvb_sb[:], bvb)
        for st in range(NST):
            va = vaug[st][:].rearrange("p (h e) -> p h e", h=HG)
            nc.gpsimd.memset(va[:, :, DK], 1.0)
        for op in ops:  # block 0 projections run up front
            op()

        for j in range(NSB):
            # anything due by this block runs now (normally already drained)
            while filler and filler[0][0] <= j:
                filler.pop(0)[1]()
            if j + 1 < NSB:
                nqt, ops = emit_proj(j + 1)
                filler.extend((j + 1, op) for op in ops)
            else:
                nqt = None

            ot_pair = [sbig.tile([128, SB], F16, tag=f"ot{c}", name=f"ot{j}_{c}",
                                 bufs=2) for c in range(2)]

            for c in range(2):
                pvA = ps_pv.tile([128, SB], F32, tag="pv", name="pvA")
                pvB = ps_pv.tile([128, SB], F32, tag="pv", name="pvB")
                n_tiles = 4 * j + 4
                for i in range(n_tiles):
                    kb, t = divmod(i, 4)
                    m = i - 4 * j
                    # q columns < c0 are entirely above the causal diagonal
                    c0 = 128 * m if (SLICED and m > 0) else 0
                    kt = kth[c][kb]
                    sc = ps_big.tile([128, 2 * SB], F32, tag="sc", name="sc")
                    nc.tensor.matmul(sc[:, c0:SB],
                                     kt[0:64, 128 * t:128 * t + 128],
                                     qt_pair[c][0:64, c0:SB],
                                     start=True, stop=True)
                    nc.tensor.matmul(sc[:, SB + c0:2 * SB],
                                     kt[64:128, 128 * t:128 * t + 128],
                                     qt_pair[c][64:128, c0:SB],
                                     start=True, stop=True)
                    e = epool.tile([128, 2 * SB], F16, tag="e", name="e")
                    ev = e[:].rearrange("p (g f) -> p g f", g=2)
                    scv = sc[:].rearrange("p (g f) -> p g f", g=2)
                    nc.scalar.activation(ev[:, :, c0:SB], scv[:, :, c0:SB],
                                         mybir.ActivationFunctionType.Exp,
                                         scale=SCALE)
                    if m >= 0:  # diagonal tile: zero the triangular band
                        band = ev[:, :, c0:128 * (m + 1)]
                        nc.gpsimd.affine_select(
                            out=band, in_=band,
                            compare_op=mybir.AluOpType.is_ge,
                            fill=0.0, base=c0 - 128 * m,
                            pattern=[[0, 2], [1, 128 * (m + 1) - c0]],
                            channel_multiplier=-1)
                    if dbg and j == 0 and c == 0 and i == 0:
                        nc.sync.dma_start(dbg_t["dbg_e"][:], e[:])
                    va = vaug[i][:].rearrange("p (h e) -> p h e", h=HG)
                    nc.tensor.matmul(pvA[0:DK + 1, c0:SB],
                                     va[:, 2 * c, 0:DK + 1], e[:, c0:SB],
                                     start=(i == 0), stop=(i == n_tiles - 1))
                    nc.tensor.matmul(pvB[0:DK + 1, c0:SB],
                                     va[:, 2 * c + 1, 0:DK + 1],
                                     e[:, SB + c0:2 * SB],
                                     start=(i == 0), stop=(i == n_tiles - 1))
                    # drain one filler PE group per key tile
                    if filler:
                        filler.pop(0)[1]()
                # normalize: ot rows = pv[0:64] * broadcast(1/pv[64])
                for half, pv in ((0, pvA), (1, pvB)):
                    # custom-DVE ops misread PSUM at base_partition 64 on HW:
                    # stage the denominator row to SBUF partition 0 first
                    dsb = npool.tile([1, SB], F32, tag="dsb", name="dsb")
                    nc.vector.tensor_copy(dsb[0:1, :], pv[DK:DK + 1, :])
                    rec = npool.tile([1, SB], F32, tag="rec", name="rec")
                    nc.vector.reciprocal_approx_fast(rec[0:1, :], dsb[0:1, :])
                    rbc = npool.tile([DK, SB], F32, tag="rbc", name="rbc")
                    nc.gpsimd.partition_broadcast(rbc[0:DK, :], rec[0:1, :])
                    rows = slice(DK * half, DK * half + DK)
                    if dbg and j == 0 and c == 0 and half == 0:
                        den = npool.tile([1, SB], F32, tag="den", name="den")
                        nc.vector.tensor_copy(den[0:1, :], pv[DK:DK + 1, :])
                        nc.sync.dma_start(dbg_t["dbg_den"][0:1, :], den[0:1, :])
                        nc.sync.dma_start(dbg_t["dbg_rec"][0:1, :], rec[0:1, :])
                        nc.sync.dma_start(dbg_t["dbg_rbc"][0:DK, :], rbc[0:DK, :])
                    nc.vector.tensor_mul(ot_pair[c][rows, :], pv[0:DK, :],
                                         rbc[0:DK, :])
                if dbg and j == 0 and c == 0:
                    nc.sync.dma_start(dbg_t["dbg_qt"][:], qt_pair[0][:])
                    nc.sync.dma_start(dbg_t["dbg_kt"][:], kth[0][0][:])
                    nc.sync.dma_start(dbg_t["dbg_va"][:], vaug[0][:])
                    nc.sync.dma_start(dbg_t["dbg_ot"][:], ot_pair[0][:])

            # Wo for this block; on the last block run immediately, else queue
            wops = emit_wo(j, ot_pair)
            if j + 1 == NSB:
                for _, op in filler:
                    op()
                for op in wops:
                    op()
                filler = []
            else:
                filler.extend((j + 2, op) for op in wops)
            qt_pair = nqt

    nc.compile()
    return nc


_CACHED_NC = {}


def _get_program(with_bias=False):
    if with_bias not in _CACHED_NC:
        _CACHED_NC[with_bias] = build_program(with_bias=with_bias)
    return _CACHED_NC[with_bias]


# ---------------------------------------------------------------------------
# entry point
# ---------------------------------------------------------------------------

def kernel(x, token_position, Wq, bq, Wk, bk, Wv, bv, Wo, bo, _results=None):
    from concourse.bass_utils import run_bass_kernel_spmd

    in_maps, with_bias = make_core_inputs(
        x, token_position, Wq, bq, Wk, bk, Wv, bv, Wo, bo)
    if _results is None:
        nc = _get_program(with_bias=with_bias)
        res = run_bass_kernel_spmd(nc, in_maps, list(range(N_CORES)))
        _results = [res.results[i]["out"] for i in range(N_CORES)]
    bo = np.asarray(bo, dtype=np.float32)
    out = np.empty((B, S, D), dtype=np.float32)
    for b in range(B):
        acc = _results[HG * b].astype(np.float32)
        for hg in range(1, HG):
            acc = acc + _results[HG * b + hg].astype(np.float32)
        out[b] = acc + bo[None, :]
    return out


# revision 35
# speedup vs baseline: 1.0276x; 1.0168x over previous
"""Trainium2 Bass kernel: causal multi-head attention with interleaved RoPE.

Problem shapes (hardcoded): x [2, 2048, 1024], 16 heads of dk=64.
Sharding: 8 cores = 2 batches x 4 head-groups (4 heads each). Each core
computes its head-slice Q/K/V projections, RoPE, causal attention, and a
partial output through its Wo row-slice; the host sums the 4 partials per
batch and adds bo.

v2 design (single interleaved pipeline, fp16 operands):
- All matmul operands fp16 (tolerance 2e-2; fp16 keeps ~1e-3).
- Heads processed as 2 pairs per core. Scores for a pair run as two
  concurrent row-tiled matmuls (tile_position rows 0-63 / 64-127) writing
  two PSUM banks in one 512-cycle pass -- 2x score throughput vs padded-K.
- Wo contracts a stacked head-pair (128 rows), halving Wo matmuls.
- One loop over the 4 sq-blocks interleaves projection(j+1) / Wo(j) matmul
  groups into the ACT-bound attention(j) stream so the PE never drains.
- Causal: only the needed q-column range of each diagonal score tile is
  exp'd / PV'd; triangular masking only on the 128-col diagonal band.
- Softmax denominator rides as a 65th "ones" column of V (PV psum row 64);
  reciprocal via DVE reciprocal_approx_fast, broadcast via gpsimd
  partition_broadcast (no PE/PSUM involvement).

RoPE trick: attention scores are invariant to any permutation of the dk
axis applied to both Q and K, so the Wq/Wk columns are permuted on the host
into a "quadrant half-split" layout where each rotation pair partner sits
exactly 16 partitions away inside the same 32-partition quadrant. The DVE
stream_shuffle (a per-quadrant 32-way permute) then produces the swapped
operand, and RoPE becomes: rot = q * cosT + shuffle(q) * sinT with
host-precomputed tables (sinT carries the sign).
"""

from contextlib import ExitStack

import numpy as np

import concourse.bass as bass
import concourse.mybir as mybir
import concourse.tile as tile

B, S, D, H = 2, 2048, 1024, 16
DK = D // H  # 64
HG = 4  # heads per core
NCOLS = HG * DK  # 256 columns of the projection per core
THETA = 10000.0
SCALE = 1.0 / float(np.sqrt(DK))
N_CORES = 8

F32 = mybir.dt.float32
F16 = mybir.dt.float16

SB = 512            # sq block width
NSB = S // SB       # 4
NST = S // 128      # 16 key tiles
NDC = D // 128      # 8 contraction chunks
AUGW = 72           # per-head stride in vaug (64 V cols + ones col + pad)
SLICED = True       # restrict diagonal score tiles to the causal q-cols


def f16(a):
    return np.ascontiguousarray(a, dtype=np.float16)


# ---------------------------------------------------------------------------
# host-side prep
# ---------------------------------------------------------------------------

def _rope_perm():
    """Within-head column permutation pi: new row r -> original dk index."""
    perm = np.empty(DK, dtype=np.int64)
    for r in range(DK):
        q, m = divmod(r, 32)
        if m < 16:
            perm[r] = 2 * (16 * q + m)
        else:
            perm[r] = 2 * (16 * q + m - 16) + 1
    return perm


_PERM = _rope_perm()
SHUF_MASK = list(range(16, 32)) + list(range(16))  # swap 16-halves per quadrant


def _shuf128(v):
    """Apply the quadrant shuffle to a length-128 vector (host mirror)."""
    out = np.empty_like(v)
    for q in range(4):
        out[32 * q:32 * q + 32] = v[32 * q:32 * q + 32][SHUF_MASK]
    return out


def _rope_tables(pos):
    """cosT/sinT [128, S] for the permuted layout. pos: [S] int."""
    inv_freq = (np.float32(THETA) ** (-(np.arange(0, DK, 2, dtype=np.float32) / np.float32(DK))))  # [32]
    ang = pos.astype(np.float32)[:, None] * inv_freq[None, :]  # [S, 32]
    cos = np.cos(ang)  # [S, 32]
    sin = np.sin(ang)
    cosT = np.empty((128, S), dtype=np.float32)
    sinT = np.empty((128, S), dtype=np.float32)
    for p in range(128):
        r = p % DK
        q, m = divmod(r, 32)
        if m < 16:
            i = 16 * q + m
            sgn = -1.0
        else:
            i = 16 * q + m - 16
            sgn = 1.0
        cosT[p] = cos[:, i]
        sinT[p] = np.float32(sgn) * sin[:, i]
    return cosT, sinT


def make_core_inputs(x, token_position, Wq, bq, Wk, bk, Wv, bv, Wo, bo):
    """Build the 8 per-core input maps. Returns (in_maps, with_bias)."""
    x = np.asarray(x, dtype=np.float32)
    token_position = np.asarray(token_position)
    Wq, Wk, Wv, Wo = (np.asarray(w, dtype=np.float32) for w in (Wq, Wk, Wv, Wo))
    bq, bk, bv = (np.asarray(b_, dtype=np.float32) for b_ in (bq, bk, bv))
    with_bias = any(float(np.abs(v).max()) != 0.0 for v in (bq, bk, bv))

    in_maps = []
    tables = {}
    for c in range(N_CORES):
        b, hg = divmod(c, HG)
        heads = range(HG * hg, HG * hg + HG)
        # permuted q/k column indices for this core's heads
        cols_qk = np.concatenate([DK * h + _PERM for h in heads])
        cols_v = np.arange(NCOLS * hg, NCOLS * hg + NCOLS)
        if b not in tables:
            tables[b] = _rope_tables(np.asarray(token_position[b]))
        cosT, sinT = tables[b]
        # Wo rows stacked per head pair: wo[r, p, :] = Wo row of
        # (local head 2p + r//64, dk r%64)
        wo_rows = Wo[cols_v, :].reshape(2, 2 * DK, D)  # [pair, 128, D]
        # weights/x pre-laid in SBUF layout [128, dc, cols] so each tensor
        # loads with a single large DMA (per-DMA queue issue is ~650ns)
        def dc_layout(w):  # [1024, C] -> [128, 8, C]
            return f16(w.reshape(NDC, 128, -1).transpose(1, 0, 2))
        # x block-major: [NSB, 128, NDC, SB] so per-block loads are contiguous
        xb = x[b].T.reshape(NDC, 128, NSB, SB).transpose(2, 1, 0, 3)
        m = {
            "xT": f16(xb),                                   # [4, 128, 8, 512]
            "wq": dc_layout(Wq[:, cols_qk]),                 # [128, 8, 256]
            "wk": dc_layout(Wk[:, cols_qk]),
            "wv": dc_layout(Wv[:, cols_v]),
            "wo": f16(wo_rows.transpose(1, 0, 2)),           # [128, 2, 1024]
            "cosT": f16(cosT),
            "sinT": f16(sinT),
        }
        if with_bias:
            # rope is linear: rope(q + b) = rope(q) + rope(b); rope(b) is a
            # position-dependent table added after the rope combine.
            for nm, bb in (("bqr", bq[cols_qk]), ("bkr", bk[cols_qk])):
                chunks = []
                for half in range(2):
                    blk = bb[128 * half:128 * half + 128]  # [128]
                    tbl = blk[:, None] * cosT + _shuf128(blk)[:, None] * sinT
                    chunks.append(f16(tbl))
                m[nm] = np.stack(chunks, axis=0)  # [2, 128, S]
            m["bvb"] = f16(np.broadcast_to(bv[cols_v][None, :], (128, NCOLS)).copy())
        in_maps.append(m)
    return in_maps, with_bias


# ---------------------------------------------------------------------------
# device program
# ---------------------------------------------------------------------------

def build_program(with_bias=False, dbg=False):
    from concourse import bacc, library_config
    nc = bacc.Bacc("TRN2", debug=False)

    xT = nc.declare_dram_parameter("xT", [NSB, 128, NDC, SB], F16,
                                   isOutput=False).ap()
    wq = nc.declare_dram_parameter("wq", [128, NDC, NCOLS], F16, isOutput=False).ap()
    wk = nc.declare_dram_parameter("wk", [128, NDC, NCOLS], F16, isOutput=False).ap()
    wv = nc.declare_dram_parameter("wv", [128, NDC, NCOLS], F16, isOutput=False).ap()
    wo = nc.declare_dram_parameter("wo", [128, 2, D], F16, isOutput=False).ap()
    cosT = nc.declare_dram_parameter("cosT", [128, S], F16, isOutput=False).ap()
    sinT = nc.declare_dram_parameter("sinT", [128, S], F16, isOutput=False).ap()
    if with_bias:
        bqr = nc.declare_dram_parameter("bqr", [2, 128, S], F16, isOutput=False).ap()
        bkr = nc.declare_dram_parameter("bkr", [2, 128, S], F16, isOutput=False).ap()
        bvb = nc.declare_dram_parameter("bvb", [128, NCOLS], F16, isOutput=False).ap()
    out = nc.declare_dram_parameter("out", [S, D], F16, isOutput=True).ap()
    if dbg:
        dbg_t = {}
        for nm, shp, dt in (("dbg_qt", [128, SB], F16), ("dbg_kt", [128, SB], F16),
                            ("dbg_va", [128, HG * AUGW], F16),
                            ("dbg_e", [128, 2 * SB], F16),
                            ("dbg_den", [128, SB], F32),
                            ("dbg_rec", [128, SB], F32),
                            ("dbg_rbc", [128, SB], F32),
                            ("dbg_ot", [128, SB], F16)):
            dbg_t[nm] = nc.declare_dram_parameter(nm, shp, dt, isOutput=True).ap()

    with tile.TileContext(nc) as tc, ExitStack() as ctx:
        nc.gpsimd.load_library(library_config.proxy)
        const = ctx.enter_context(tc.tile_pool(name="const", bufs=1))
        sbig = ctx.enter_context(tc.tile_pool(name="sbig", bufs=1))
        xts = ctx.enter_context(tc.tile_pool(name="xts", bufs=2))
        rtmp = ctx.enter_context(tc.tile_pool(name="rtmp", bufs=2))
        epool = ctx.enter_context(tc.tile_pool(name="epool", bufs=3))
        npool = ctx.enter_context(tc.tile_pool(name="npool", bufs=2))
        opool = ctx.enter_context(tc.tile_pool(name="opool", bufs=3))
        ps_big = ctx.enter_context(tc.tile_pool(name="ps_big", bufs=2, space="PSUM"))
        ps_pv = ctx.enter_context(tc.tile_pool(name="ps_pv", bufs=2, space="PSUM"))
        ps_pj = ctx.enter_context(tc.tile_pool(name="ps_pj", bufs=2, space="PSUM"))

        # --- weights / tables resident in SBUF, one large DMA per tensor
        # (per-DMA queue issue costs ~650ns; batching beats fine deps).
        # Priority order: wk/wq, x block 0 (issued in emit_proj(0) below),
        # wv, block-0 rope table columns, the rest.
        wq_sb = const.tile([128, NDC, NCOLS], F16, tag="wq")
        wk_sb = const.tile([128, NDC, NCOLS], F16, tag="wk")
        wv_sb = const.tile([128, NDC, NCOLS], F16, tag="wv")
        # (wk is DMA'd per-dc interleaved with block-0 x inside emit_proj)
        # HAM warm-up: dep-free junk matmuls keep the PE clock at 2.4 GHz
        # through the initial DMA fill (idle >3.4us re-throttles to 1.2 GHz)
        warm = const.tile([128, SB], F16, tag="warm")
        wps = ps_pj.tile([128, SB], F32, tag="pj", name="warmps")
        with tc.high_priority():
            nc.gpsimd.memset(warm[:], 0.0)
            for w in range(16):
                nc.tensor.matmul(wps[:], warm[:, 0:128], warm[:],
                                 start=(w == 0), stop=(w == 15))
        cos_sb = const.tile([128, S], F16, tag="cos")
        sin_sb = const.tile([128, S], F16, tag="sin")
        wo_sb = const.tile([128, 2, D], F16, tag="wo")
        if with_bias:
            bqr_t = [const.tile([128, S], F16, tag=f"bqr{c}", name=f"bqr{c}")
                     for c in range(2)]
            bkr_t = [const.tile([128, S], F16, tag=f"bkr{c}", name=f"bkr{c}")
                     for c in range(2)]
            bvb_sb = const.tile([128, NCOLS], F16, tag="bvb")

        # rope'd K^T per (chunk, block); resident for the whole sequence
        kth = [[sbig.tile([128, SB], F16, tag=f"kh{c}_{sb}", name=f"kh{c}_{sb}")
                for sb in range(NSB)] for c in range(2)]
        # V augmented with a ones column per head, per key tile
        vaug = [sbig.tile([128, HG * AUGW], F16, tag=f"va{st}", name=f"va{st}")
                for st in range(NST)]

        # ------------------------------------------------------------------
        # helpers
        # ------------------------------------------------------------------

        def emit_proj(j):
            """Allocate block-j projection outputs and return (qt_pair, ops):
            ops is a list of closures, each issuing one PE matmul group plus
            its vector consumers."""
            ss = slice(SB * j, SB * j + SB)
            xt_t = xts.tile([128, NDC, SB], F16, tag="xt", name=f"xt{j}")
            if j == 0:
                # fine-grained per-dc loads interleaved with wk so the first
                # projection matmul starts after ~192KB of DMA
                for dc in range(NDC):
                    nc.sync.dma_start(wk_sb[:, dc, :], wk[:, dc, :])
                    nc.sync.dma_start(xt_t[:, dc, :], xT[0, :, dc, :])
            else:
                nc.sync.dma_start(xt_t[:], xT[j])
            # rope'd Q^T pair tiles for this block (2-buf rotation)
            qt_pair = [sbig.tile([128, SB], F16, tag=f"qt{c}", name=f"qt{j}_{c}",
                                 bufs=2) for c in range(2)]

            ops = []

            def qk_group(c, kind):
                def run():
                    w_sb = wq_sb if kind == "q" else wk_sb
                    ncol = slice(128 * c, 128 * c + 128)
                    ps = ps_pj.tile([128, SB], F32, tag="pj", name="psqk")
                    for dc in range(NDC):
                        nc.tensor.matmul(ps[:], w_sb[:, dc, ncol],
                                         xt_t[:, dc, :],
                                         start=(dc == 0), stop=(dc == NDC - 1))
                    # rope: dst = ps*cos + shuffle(ps)*sin (+ bias table)
                    t_cos = rtmp.tile([128, SB], F16, tag="rc", name="tcos")
                    nc.vector.tensor_mul(t_cos[:], ps[:], cos_sb[:, ss])
                    t_shuf = rtmp.tile([128, SB], F32, tag="rs", name="tshuf")
                    nc.vector.stream_shuffle(t_shuf[:], ps[:], SHUF_MASK)
                    t_sin = rtmp.tile([128, SB], F16, tag="rm", name="tsin")
                    nc.gpsimd.tensor_mul(t_sin[:], t_shuf[:], sin_sb[:, ss])
                    dst = qt_pair[c] if kind == "q" else kth[c][j]
                    nc.vector.tensor_add(dst[:], t_cos[:], t_sin[:])
                    if with_bias:
                        bt = bqr_t[c] if kind == "q" else bkr_t[c]
                        nc.vector.tensor_add(dst[:], dst[:], bt[:, ss])
                return run

            def v_group(st4):
                def run():
                    st = 4 * j + st4
                    ps = ps_pj.tile([128, SB], F32, tag="pj", name="psv")
                    for dc in range(NDC):
                        nc.tensor.matmul(ps[:, 0:NCOLS],
                                         xt_t[:, dc, 128 * st4:128 * st4 + 128],
                                         wv_sb[:, dc, :],
                                         start=(dc == 0), stop=(dc == NDC - 1))
                    va = vaug[st][:].rearrange("p (h e) -> p h e", h=HG)
                    psv = ps[:, 0:NCOLS].rearrange("p (h k) -> p h k", h=HG)
                    if with_bias:
                        bvv = bvb_sb[:].rearrange("p (h k) -> p h k", h=HG)
                        nc.vector.tensor_add(va[:, :, 0:DK], psv, bvv)
                    else:
                        nc.vector.tensor_copy(va[:, :, 0:DK], psv)
                return run

            # chunk-0 attention deps first, then V, then chunk 1
            ops.append(qk_group(0, "k"))
            ops.append(qk_group(0, "q"))
            for st4 in range(4):
                ops.append(v_group(st4))
            ops.append(qk_group(1, "k"))
            ops.append(qk_group(1, "q"))
            return qt_pair, ops

        def emit_wo(j, ot_pair):
            """Wo matmul groups for block j (reads the ot pair tiles)."""
            def grp(rq):
                def run():
                    ps = ps_big.tile([128, 2 * SB], F32, tag="sc", name="pswo")
                    for half in range(2):
                        cols = slice(SB * half, SB * half + SB)
                        nc.tensor.matmul(ps[:, cols],
                                         ot_pair[0][:, 128 * rq:128 * rq + 128],
                                         wo_sb[:, 0, cols], start=True, stop=False)
                        nc.tensor.matmul(ps[:, cols],
                                         ot_pair[1][:, 128 * rq:128 * rq + 128],
                                         wo_sb[:, 1, cols], start=False, stop=True)
                    o_sb = opool.tile([128, 2 * SB], F16, tag="osb", name="osb")
                    nc.vector.tensor_copy(o_sb[:], ps[:])
                    r0 = SB * j + 128 * rq
                    nc.sync.dma_start(out[r0:r0 + 128, :], o_sb[:])
                return run

            return [grp(rq) for rq in range(4)]

        # ------------------------------------------------------------------
        # main pipeline over sq blocks
        # ------------------------------------------------------------------

        # filler: list of (due_block, closure). A filler op must have run
        # before the attention of block `due_block` is emitted.
        filler = []

        qt_pair, ops = emit_proj(0)  # also issues block-0 wk + x DMAs
        # block-0 critical path next: wq, wv, block-0 rope tables
        nc.sync.dma_start(wq_sb[:], wq)
        nc.sync.dma_start(wv_sb[:], wv)
        nc.sync.dma_start(cos_sb[:, 0:SB], cosT[:, 0:SB])
        nc.sync.dma_start(sin_sb[:, 0:SB], sinT[:, 0:SB])
        # remaining table columns + Wo afterwards
        nc.sync.dma_start(cos_sb[:, SB:S], cosT[:, SB:S])
        nc.sync.dma_start(sin_sb[:, SB:S], sinT[:, SB:S])
        nc.sync.dma_start(wo_sb[:], wo)
        if with_bias:
            for c in range(2):
                nc.sync.dma_start(bqr_t[c][:], bqr[c])
                nc.sync.dma_start(bkr_t[c][:], bkr[c])
            nc.sync.dma_start(bvb_sb[:], bvb)
        for st in range(NST):
            va = vaug[st][:].rearrange("p (h e) -> p h e", h=HG)
            nc.gpsimd.memset(va[:, :, DK], 1.0)
        for op in ops:  # block 0 projections run up front
            op()

        for j in range(NSB):
            # anything due by this block runs now (normally already drained)
            while filler and filler[0][0] <= j:
                filler.pop(0)[1]()
            if j + 1 < NSB:
                nqt, ops = emit_proj(j + 1)
                filler.extend((j + 1, op) for op in ops)
            else:
                nqt = None

            ot_pair = [sbig.tile([128, SB], F16, tag=f"ot{c}", name=f"ot{j}_{c}",
                                 bufs=2) for c in range(2)]

            for c in range(2):
                pvA = ps_pv.tile([128, SB], F32, tag="pv", name="pvA")
                pvB = ps_pv.tile([128, SB], F32, tag="pv", name="pvB")
                n_tiles = 4 * j + 4
                for i in range(n_tiles):
                    kb, t = divmod(i, 4)
                    m = i - 4 * j
                    # q columns < c0 are entirely above the causal diagonal
                    c0 = 128 * m if (SLICED and m > 0) else 0
                    kt = kth[c][kb]
                    sc = ps_big.tile([128, 2 * SB], F32, tag="sc", name="sc")
                    nc.tensor.matmul(sc[:, c0:SB],
                                     kt[0:64, 128 * t:128 * t + 128],
                                     qt_pair[c][0:64, c0:SB],
                                     start=True, stop=True)
                    nc.tensor.matmul(sc[:, SB + c0:2 * SB],
                                     kt[64:128, 128 * t:128 * t + 128],
                                     qt_pair[c][64:128, c0:SB],
                                     start=True, stop=True)
                    e = epool.tile([128, 2 * SB], F16, tag="e", name="e")
                    ev = e[:].rearrange("p (g f) -> p g f", g=2)
                    scv = sc[:].rearrange("p (g f) -> p g f", g=2)
                    nc.scalar.activation(ev[:, :, c0:SB], scv[:, :, c0:SB],
                                         mybir.ActivationFunctionType.Exp,
                                         scale=SCALE)
                    if m >= 0:  # diagonal tile: zero the triangular band
                        band = ev[:, :, c0:128 * (m + 1)]
                        nc.gpsimd.affine_select(
                            out=band, in_=band,
                            compare_op=mybir.AluOpType.is_ge,
                            fill=0.0, base=c0 - 128 * m,
                            pattern=[[0, 2], [1, 128 * (m + 1) - c0]],
                            channel_multiplier=-1)
                    if dbg and j == 0 and c == 0 and i == 0:
                        nc.sync.dma_start(dbg_t["dbg_e"][:], e[:])
                    va = vaug[i][:].rearrange("p (h e) -> p h e", h=HG)
                    nc.tensor.matmul(pvA[0:DK + 1, c0:SB],
                                     va[:, 2 * c, 0:DK + 1], e[:, c0:SB],
                                     start=(i == 0), stop=(i == n_tiles - 1))
                    nc.tensor.matmul(pvB[0:DK + 1, c0:SB],
                                     va[:, 2 * c + 1, 0:DK + 1],
                                     e[:, SB + c0:2 * SB],
                                     start=(i == 0), stop=(i == n_tiles - 1))
                    # drain one filler PE group per key tile
                    if filler:
                        filler.pop(0)[1]()
                # normalize: ot rows = pv[0:64] * broadcast(1/pv[64])
                for half, pv in ((0, pvA), (1, pvB)):
                    # custom-DVE ops misread PSUM at base_partition 64 on HW:
                    # stage the denominator row to SBUF partition 0 first
                    dsb = npool.tile([1, SB], F32, tag="dsb", name="dsb")
                    nc.vector.tensor_copy(dsb[0:1, :], pv[DK:DK + 1, :])
                    rec = npool.tile([1, SB], F32, tag="rec", name="rec")
                    nc.vector.reciprocal_approx_fast(rec[0:1, :], dsb[0:1, :])
                    rbc = npool.tile([DK, SB], F32, tag="rbc", name="rbc")
                    nc.gpsimd.partition_broadcast(rbc[0:DK, :], rec[0:1, :])
                    rows = slice(DK * half, DK * half + DK)
                    if dbg and j == 0 and c == 0 and half == 0:
                        den = npool.tile([1, SB], F32, tag="den", name="den")
                        nc.vector.tensor_copy(den[0:1, :], pv[DK:DK + 1, :])
                        nc.sync.dma_start(dbg_t["dbg_den"][0:1, :], den[0:1, :])
                        nc.sync.dma_start(dbg_t["dbg_rec"][0:1, :], rec[0:1, :])
                        nc.sync.dma_start(dbg_t["dbg_rbc"][0:DK, :], rbc[0:DK, :])
                    nc.vector.tensor_mul(ot_pair[c][rows, :], pv[0:DK, :],
                                         rbc[0:DK, :])
                if dbg and j == 0 and c == 0:
                    nc.sync.dma_start(dbg_t["dbg_qt"][:], qt_pair[0][:])
                    nc.sync.dma_start(dbg_t["dbg_kt"][:], kth[0][0][:])
                    nc.sync.dma_start(dbg_t["dbg_va"][:], vaug[0][:])
                    nc.sync.dma_start(dbg_t["dbg_ot"][:], ot_pair[0][:])

            # Wo for this block; on the last block run immediately, else queue
            wops = emit_wo(j, ot_pair)
            if j + 1 == NSB:
                for _, op in filler:
                    op()
                for op in wops:
                    op()
                filler = []
            else:
                filler.extend((j + 2, op) for op in wops)
            qt_pair = nqt

    nc.compile()
    return nc


_CACHED_NC = {}


def _get_program(with_bias=False):
    if with_bias not in _CACHED_NC:
        _CACHED_NC[with_bias] = build_program(with_bias=with_bias)
    return _CACHED_NC[with_bias]


# ---------------------------------------------------------------------------
# entry point
# ---------------------------------------------------------------------------

def kernel(x, token_position, Wq, bq, Wk, bk, Wv, bv, Wo, bo, _results=None):
    from concourse.bass_utils import run_bass_kernel_spmd

    in_maps, with_bias = make_core_inputs(
        x, token_position, Wq, bq, Wk, bk, Wv, bv, Wo, bo)
    if _results is None:
        nc = _get_program(with_bias=with_bias)
        res = run_bass_kernel_spmd(nc, in_maps, list(range(N_CORES)))
        _results = [res.results[i]["out"] for i in range(N_CORES)]
    bo = np.asarray(bo, dtype=np.float32)
    out = np.empty((B, S, D), dtype=np.float32)
    for b in range(B):
        acc = _results[HG * b].astype(np.float32)
        for hg in range(1, HG):
            acc = acc + _results[HG * b + hg].astype(np.float32)
        out[b] = acc + bo[None, :]
    return out
